# revision 19
# baseline (speedup 1.0000x reference)
"""CrossAttention kernel for 8 Trainium2 NeuronCores (Bass/Tile).

Sharding: tensor-parallel over heads. Core i handles heads {2i, 2i+1} for
both batch elements (128 channels).

v2 design notes (vs the v1 baseline):
- alibi is exponentiated on the host: ea = exp(alibi) in bf16. Device-side
  the softmax becomes exp(scores) * ea -- one bf16 DVE multiply per score
  tile (2x DVE mode) instead of f32 adds + PE identity matmuls, and the
  alibi HBM traffic halves (bf16 instead of f32).
- Projections are post-scaled: ps = W_s@x_raw - mu (x) wbar accumulates in
  PSUM (raw, unnormalized rhs), then one Pool-engine multiply by the
  broadcast 1/sigma applies the LN scale. No per-chunk input scaling.
- LN stats: x and x^2 streams are tree-folded 8->2 chunks on DVE (bf16 2x),
  then a onehot ones-matmul on PE reduces the remaining 2x128 channels,
  accumulating all token tiles into one [4, TT] PSUM tile per stat.
- V is built directly in [key, dh] natural layout by flipping the matmul
  (lhsT = cT token block, rhs = Wv chunk), so no PE transposes / vaug
  copies; the 1/sigma scale rides the PSUM->SBUF Act copy as a per-key
  scale vector (obtained by tiny PE transposes of the stat rows).
- The two heads' scores go into one 2-bank PSUM tile so one Act exp
  covers [128, 1024]; the softmax denominator rides the AV matmul as a
  ones-column of V (row 64 of the 65-row AV output).
- Output projection PSUM is staged to SBUF by the (otherwise idle) Pool
  engine; bo is added on the host during the gather.
Host gather: sum the 8 partial [dout, tok] projections, add bo, transpose.
"""

import os
import sys

for _p in ("/opt/trn_rl_repo", "/root/.axon_site/_ro/trn_rl_repo"):
    if os.path.isdir(_p) and _p not in sys.path:
        sys.path.insert(0, _p)

import numpy as np
import ml_dtypes

import concourse.bass as bass
import concourse.tile as tile
from concourse import bacc, mybir
from concourse.masks import make_identity

BF16 = ml_dtypes.bfloat16

HEADS = 16
N_CORES = 8
H_PER_CORE = HEADS // N_CORES  # 2
DH = 64
LN_EPS = 1e-5

B = 2
N_TOK = 2048
D = 1024

QT = 512            # query tile (free dim of scores matmuls)
KT = 128            # key tile (partition dim of scoresT)
TT = 512            # token tile for LN/projection phase
N_DT = D // 128     # 8 contraction tiles of 128 over d


def build_program(n_tok=N_TOK, with_pbias=False):
    """Build the single-core SPMD Bass program. Returns nc."""
    nc = bacc.Bacc("TRN2")
    f32 = mybir.dt.float32
    f32r = mybir.dt.float32r
    bf16 = mybir.dt.bfloat16
    AF = mybir.ActivationFunctionType
    ALU = mybir.AluOpType

    n_tt = n_tok // TT          # token tiles per batch
    n_qt = n_tok // QT          # query tiles per batch
    n_kt = n_tok // KT          # key tiles per batch

    # ---- DRAM parameters (per-core shards, host-prepped) ----
    xT = nc.declare_dram_parameter("xT", [B, D, n_tok], bf16, isOutput=False)
    cT = nc.declare_dram_parameter("cT", [B, D, n_tok], bf16, isOutput=False)
    # exp(alibi) transposed: [h, key, q], bf16
    eaT = nc.declare_dram_parameter(
        "eaT", [H_PER_CORE, n_tok, n_tok], bf16, isOutput=False)
    wqT = nc.declare_dram_parameter("wqT", [D, 128], bf16, isOutput=False)
    wkT = nc.declare_dram_parameter("wkT", [D, 128], bf16, isOutput=False)
    wvT = nc.declare_dram_parameter("wvT", [D, 128], bf16, isOutput=False)
    # rows: -wbar_q, -wbar_k, -wbar_v   (sum over d of the scaled weights)
    wbar = nc.declare_dram_parameter("wbar", [3, 128], bf16, isOutput=False)
    woT = nc.declare_dram_parameter("woT", [128, D], bf16, isOutput=False)
    # host-computed LN stats: mean rows (bf16), 1/sigma rows (f32),
    # per-key 1/sigma columns for the exp scale (f32)
    mrow = nc.declare_dram_parameter("mrow", [2, B, n_tok], bf16, isOutput=False)
    irow = nc.declare_dram_parameter("irow", [2, B, n_tok], f32, isOutput=False)
    icol = nc.declare_dram_parameter("icol", [B, 128, n_tok // 128], f32,
                                     isOutput=False)
    if with_pbias:
        # rows: Wq@ln_b*scale, Wk@ln_b, Wv@ln_b
        pbias = nc.declare_dram_parameter("pbias", [3, 128], bf16, isOutput=False)
        srow = nc.declare_dram_parameter("srow", [2, B, n_tok], bf16,
                                         isOutput=False)

    outT = nc.declare_dram_parameter(
        "outT", [D, B * n_tok], f32, isOutput=True)

    xT_r = xT.rearrange("b (dt p) n -> b p dt n", p=128)
    cT_r = cT.rearrange("b (dt p) n -> b p dt n", p=128)
    woT_r = woT.rearrange("c (dt n) -> c dt n", n=128)
    outT_r = outT.rearrange("(dt p) n -> p dt n", p=128)

    with tile.TileContext(nc) as tc:
        with tc.tile_pool(name="const", bufs=1) as const_pool, \
             tc.tile_pool(name="rowp", bufs=2) as rowp:
            ident_b = const_pool.tile([128, 128], bf16, name="ident_b")
            make_identity(nc, ident_b)
            icol_sb = const_pool.tile([128, B, n_tok // 128], f32,
                                      name="icol_sb")
            nc.sync.dma_start(out=icol_sb,
                              in_=icol.rearrange("b p k -> p b k"))

            wq_sb = const_pool.tile([128, N_DT, 128], bf16, name="wq_sb")
            wk_sb = const_pool.tile([128, N_DT, 128], bf16, name="wk_sb")
            wv_sb = const_pool.tile([128, N_DT, 128], bf16, name="wv_sb")
            nc.sync.dma_start(out=wq_sb, in_=wqT.rearrange("(dt p) c -> p dt c", p=128))
            nc.sync.dma_start(out=wk_sb, in_=wkT.rearrange("(dt p) c -> p dt c", p=128))
            nc.sync.dma_start(out=wv_sb, in_=wvT.rearrange("(dt p) c -> p dt c", p=128))
            wbar_sb = const_pool.tile([1, 3, 128], bf16, name="wbar_sb")
            nc.sync.dma_start(out=wbar_sb, in_=wbar[None, :, :])
            wo_sb = const_pool.tile([128, N_DT, 128], bf16, name="wo_sb")
            nc.sync.dma_start(out=wo_sb, in_=woT_r)
            if with_pbias:
                pb_sb = const_pool.tile([1, 3, 128], bf16, name="pb_sb")
                nc.sync.dma_start(out=pb_sb, in_=pbias[None, :, :])

            # persistent activations: q/k transposed f32 (f32r for PE speed)
            qT_sb = const_pool.tile([128, B, n_tok], f32r, name="qT_sb")
            kT_sb = const_pool.tile([128, B, n_tok], f32r, name="kT_sb")
            vT_sb = const_pool.tile([128, B, n_tok], bf16, name="vT_sb")
            # v natural (+ones col): [key(128), b*n_kt*h, 66]
            vaug_sb = const_pool.tile(
                [128, B * n_kt * H_PER_CORE, 66], bf16, name="vaug_sb")
            nc.vector.memset(vaug_sb[:, :, 64:65], 1.0)

            def vaug_idx(b, kt, h):
                return (b * n_kt + kt) * H_PER_CORE + h


            # ============ Phase A helpers (host LN stats) ========
            raw_p = tc.alloc_tile_pool(name="raw_p", bufs=n_tt)
            rowp = tc.alloc_tile_pool(name="rowp", bufs=2)
            isb_p = tc.alloc_tile_pool(name="isb_p", bufs=3)
            ps_pool = tc.alloc_tile_pool(name="ps_pool", bufs=2, space="PSUM")

            def emit_A(src_i, b, src_r):
                """LN-apply + projections for one (src, b). Q and V are
                post-scaled by 1/sigma (DVE); K stays unnormalized -- the
                per-key 1/sigma rides the phase-B exp as its per-partition
                scale vector. V is then transposed to [key, dh] natural
                layout through the PE."""
                raws = []
                for u in range(n_tt):
                    raw = raw_p.tile([128, N_DT, TT], bf16, tag="raw",
                                     name="raw")
                    raws.append(raw)
                    nc.sync.dma_start(
                        out=raw, in_=src_r[b, :, :, u * TT:(u + 1) * TT])
                plist = ((0, wq_sb, qT_sb),) if src_i == 0 else \
                        ((1, wk_sb, kT_sb), (2, wv_sb, vT_sb))
                for u in range(n_tt):
                    t_sl = slice(u * TT, (u + 1) * TT)
                    m_row = rowp.tile([1, TT], bf16, tag="m_row",
                                      name="m_row")
                    nc.sync.dma_start(out=m_row, in_=mrow[None, src_i, b, t_sl])
                    if with_pbias:
                        s_row = rowp.tile([1, TT], bf16, tag="s_row",
                                          name="s_row")
                        nc.sync.dma_start(out=s_row,
                                          in_=srow[None, src_i, b, t_sl])
                    isb = None
                    for wi, w_sb, dst in plist:
                        if wi != 1 and isb is None:
                            i_row = rowp.tile([1, TT], f32, tag="i_row",
                                              name="i_row")
                            nc.sync.dma_start(
                                out=i_row, in_=irow[None, src_i, b, t_sl])
                            isb = isb_p.tile([128, TT], f32, tag="isb",
                                             name="isb")
                            nc.gpsimd.partition_broadcast(isb, i_row)
                        ps = ps_pool.tile([128, TT], f32, tag="ps", name="ps")
                        for dt in range(N_DT):
                            nc.tensor.matmul(
                                ps, w_sb[:, dt, :], raws[u][:, dt, :],
                                start=(dt == 0), stop=False)
                        nc.tensor.matmul(
                            ps, wbar_sb[:, wi, :], m_row,
                            start=False, stop=not with_pbias)
                        if with_pbias:
                            nc.tensor.matmul(
                                ps, pb_sb[:, wi, :], s_row,
                                start=False, stop=True)
                        dsl = dst[:, b, t_sl]
                        if wi == 1:
                            nc.scalar.activation(
                                out=dsl, in_=ps, func=AF.Copy,
                                bias=0.0, scale=1.0)
                        else:
                            nc.vector.tensor_mul(dsl, ps, isb)
                if src_i == 1:
                    for kt in range(n_kt):
                        # vt shares the ps tag (sequential after the projs)
                        vt = ps_pool.tile([128, 128], bf16, tag="ps",
                                          name="vt")
                        nc.tensor.transpose(
                            vt, vT_sb[:, b, kt * KT:(kt + 1) * KT], ident_b)
                        i0 = vaug_idx(b, kt, 0)
                        nc.scalar.activation(
                            out=vaug_sb[:, i0:i0 + 2, 0:64],
                            in_=vt.rearrange("p (h c) -> p h c", h=2),
                            func=AF.Copy, bias=0.0, scale=1.0)

            # ============ Phase B: attention, b-major with ea reuse ========
            ea_p = tc.alloc_tile_pool(name="ea_p", bufs=n_kt + 1)
            ex_p = tc.alloc_tile_pool(name="ex_p", bufs=3)
            den_p = tc.alloc_tile_pool(name="den_p", bufs=2)
            fo_p = tc.alloc_tile_pool(name="fo_p", bufs=1)
            sc_ps = tc.alloc_tile_pool(name="sc_ps", bufs=1, space="PSUM")
            av_ps = tc.alloc_tile_pool(name="av_ps", bufs=1, space="PSUM")

            state = {"fo": [None, None]}

            def emit_unit(qt, b, dt, o_sb):
                """One output-projection unit: dt-chunk matmul + copy."""
                if dt == 0:
                    state["fo"][b] = fo_p.tile(
                        [128, N_DT, QT], f32, tag=f"fo{b}", name="fo")
                fo = state["fo"][b]
                fp = sc_ps.tile([128, 2, QT], f32, tag=f"sc_k{dt % 2}",
                                name="fp")
                nc.tensor.matmul(fp[:, 0, :], wo_sb[:, dt, :], o_sb,
                                 start=True, stop=True)
                if dt % 2 == 0:
                    nc.scalar.activation(
                        out=fo[:, dt, :], in_=fp[:, 0, :],
                        func=AF.Copy, bias=0.0, scale=1.0)
                else:
                    nc.vector.tensor_copy(fo[:, dt, :], fp[:, 0, :])
                if dt == N_DT - 1:
                    nc.sync.dma_start(
                        out=outT_r[:, :, b * n_tok + qt * QT:
                                   b * n_tok + (qt + 1) * QT],
                        in_=fo)

            pending = []
            ea_tiles = [None] * n_kt

            def emit_pass(qt, b, load_ea):
                q_sl = slice(qt * QT, (qt + 1) * QT)
                av = [av_ps.tile([65, QT], f32, tag=f"av_h{h}",
                                 name=f"av{h}") for h in range(H_PER_CORE)]
                for kt in range(n_kt):
                    if pending:
                        pqt, pb, osb = pending[0]
                        if kt < N_DT:
                            emit_unit(pqt, pb, kt, osb)
                            if kt == N_DT - 1:
                                pending.pop(0)
                    k_sl = slice(kt * KT, (kt + 1) * KT)
                    if load_ea:
                        ea_tiles[kt] = ea_p.tile([128, 2, QT], bf16,
                                                 tag="ea", name="ea")
                        nc.sync.dma_start(
                            out=ea_tiles[kt],
                            in_=eaT[:, k_sl, q_sl].rearrange("h p n -> p h n"))
                    ea = ea_tiles[kt]
                    sc2 = sc_ps.tile([128, 2, QT], f32, tag=f"sc_k{kt % 2}",
                                     name="sc2")
                    for h in range(H_PER_CORE):
                        c_sl = slice(h * 64, (h + 1) * 64)
                        nc.tensor.matmul(
                            sc2[:, h, :], kT_sb[c_sl, b, k_sl],
                            qT_sb[c_sl, b, q_sl],
                            start=True, stop=True, tile_position=(h * 64, 0))
                    ex_raw = ex_p.tile([128, 2, QT], bf16, tag="ex_raw",
                                       name="ex_raw")
                    nc.scalar.activation(
                        out=ex_raw, in_=sc2, func=AF.Exp, bias=0.0,
                        scale=icol_sb[:, b, kt:kt + 1])
                    ex = ex_p.tile([128, 2, QT], bf16, tag="ex", name="ex")
                    nc.vector.tensor_mul(ex, ex_raw, ea)
                    for h in range(H_PER_CORE):
                        nc.tensor.matmul(
                            av[h], vaug_sb[:, vaug_idx(b, kt, h), 0:65],
                            ex[:, h, :],
                            start=(kt == 0), stop=(kt == n_kt - 1))
                # normalize fast (frees the av banks for the next pass)
                o_sb = den_p.tile([128, QT], bf16, tag=f"o_sb{b}",
                                  name="o_sb")
                for h in range(H_PER_CORE):
                    den = den_p.tile([1, QT], f32, tag=f"den{h}", name="den")
                    nc.scalar.activation(out=den, in_=av[h][64:65, :],
                                         func=AF.Copy, bias=0.0, scale=1.0)
                    rden = den_p.tile([1, QT], f32, tag=f"rden{h}",
                                      name="rden")
                    nc.vector.reciprocal_approx_fast(rden, den)
                    rb = den_p.tile([64, QT], f32, tag=f"rb{h}", name="rb")
                    nc.gpsimd.partition_broadcast(rb, rden)
                    nc.vector.tensor_mul(o_sb[h * 64:(h + 1) * 64, :],
                                         av[h][0:64, :], rb)
                pending.append((qt, b, o_sb))

            emit_A(0, 0, xT_r)
            emit_A(1, 0, cT_r)
            for qt in range(n_qt):
                emit_pass(qt, 0, load_ea=True)
                if qt == 0:
                    emit_A(0, 1, xT_r)
                    emit_A(1, 1, cT_r)
                emit_pass(qt, 1, load_ea=False)
            for pqt, pb, osb in pending:
                for dt in range(N_DT):
                    emit_unit(pqt, pb, dt, osb)
            for _pool in (av_ps, sc_ps, fo_p, den_p, ex_p, ea_p,
                          ps_pool, isb_p, rowp, raw_p):
                _pool.release()
    nc.compile()
    return nc


_NC_CACHE = {}


def _get_program(n_tok=N_TOK, with_pbias=False):
    key = (n_tok, with_pbias)
    if key not in _NC_CACHE:
        _NC_CACHE[key] = build_program(n_tok, with_pbias)
    return _NC_CACHE[key]


def _prep_in_maps(x, context, alibi, Wq, Wk, Wv, Wo, bo, ln_w, ln_b):
    b, n, d = x.shape
    scale = (d // HEADS) ** -0.5

    x = np.asarray(x, dtype=np.float32)
    context = np.asarray(context, dtype=np.float32)
    alibi = np.asarray(alibi, dtype=np.float32)
    Wq, Wk, Wv, Wo = (np.asarray(w, dtype=np.float32) for w in (Wq, Wk, Wv, Wo))
    ln_w = np.asarray(ln_w, dtype=np.float32)
    ln_b = np.asarray(ln_b, dtype=np.float32)

    xT = np.ascontiguousarray(x.transpose(0, 2, 1)).astype(BF16)
    cT = np.ascontiguousarray(context.transpose(0, 2, 1)).astype(BF16)
    # exp(alibi), transposed to [h, key, q], bf16
    eaT_full = np.exp(alibi[0]).transpose(0, 2, 1)

    with_pbias = bool(np.any(ln_b != 0.0))

    # LN stats on the bf16-rounded inputs (what the device streams)
    def stats(t):
        tf = np.asarray(t, dtype=np.float32)
        mu = tf.mean(-1)                                    # [b, n]
        var = tf.var(-1)
        ivs = 1.0 / np.sqrt(var + LN_EPS)
        return mu, ivs
    mu_x, iv_x = stats(xT.transpose(0, 2, 1))
    mu_c, iv_c = stats(cT.transpose(0, 2, 1))
    mrow = np.stack([mu_x, mu_c]).astype(BF16)              # [2, B, n]
    irow = np.stack([iv_x, iv_c]).astype(np.float32)        # [2, B, n]
    icol = np.ascontiguousarray(
        iv_c.reshape(b, n // 128, 128).transpose(0, 2, 1)).astype(np.float32)
    srow = np.stack([1.0 / iv_x, 1.0 / iv_c]).astype(BF16)

    in_maps = []
    for ci in range(N_CORES):
        h0 = ci * H_PER_CORE
        cs = slice(h0 * DH, (h0 + H_PER_CORE) * DH)  # this core's 128 channels

        wq_s = (Wq[cs] * ln_w[None, :]) * scale          # [128, d]
        wk_s = Wk[cs] * ln_w[None, :]
        wv_s = Wv[cs] * ln_w[None, :]
        wbar = np.stack([
            -wq_s.sum(axis=1), -wk_s.sum(axis=1), -wv_s.sum(axis=1)])

        m = {
            "xT": xT,
            "cT": cT,
            "eaT": np.ascontiguousarray(eaT_full[h0:h0 + H_PER_CORE]).astype(BF16),
            "wqT": np.ascontiguousarray(wq_s.T).astype(BF16),
            "wkT": np.ascontiguousarray(wk_s.T).astype(BF16),
            "wvT": np.ascontiguousarray(wv_s.T).astype(BF16),
            "wbar": wbar.astype(BF16),
            "woT": np.ascontiguousarray(Wo[:, cs].T).astype(BF16),
            "mrow": mrow,
            "irow": irow,
            "icol": icol,
        }
        if with_pbias:
            m["pbias"] = np.stack([
                (Wq[cs] @ ln_b) * scale, Wk[cs] @ ln_b,
                Wv[cs] @ ln_b]).astype(BF16)
            m["srow"] = srow
        in_maps.append(m)
    return in_maps, with_pbias


def _gather(results, b, n, d, bo):
    acc = np.zeros((d, b * n), dtype=np.float32)
    for r in results:
        acc += r["outT"].astype(np.float32)
    acc += np.asarray(bo, dtype=np.float32)[:, None]
    return np.ascontiguousarray(
        acc.reshape(d, b, n).transpose(1, 2, 0)).astype(np.float32)


def kernel(**inputs):
    from concourse.bass_utils import run_bass_kernel_spmd
    x = inputs["x"]
    b, n, d = x.shape
    in_maps, with_pbias = _prep_in_maps(**inputs)
    nc = _get_program(n, with_pbias)
    res = run_bass_kernel_spmd(nc, in_maps, list(range(N_CORES)))
    return _gather(res.results, b, n, d, inputs["bo"])


def run_profiled(inputs, trace=True):
    from concourse.bass_utils import run_bass_kernel_spmd
    x = inputs["x"]
    b, n, d = x.shape
    in_maps, with_pbias = _prep_in_maps(**inputs)
    nc = _get_program(n, with_pbias)
    res = run_bass_kernel_spmd(nc, in_maps, list(range(N_CORES)), trace=trace)
    return _gather(res.results, b, n, d, inputs["bo"]), res


# revision 23
# speedup vs baseline: 1.0867x; 1.0867x over previous
"""CrossAttention kernel for 8 Trainium2 NeuronCores (Bass/Tile).

Sharding: tensor-parallel over heads. Core i handles heads {2i, 2i+1} for
both batch elements (128 channels).

v2 design notes (vs the v1 baseline):
- alibi is exponentiated on the host: ea = exp(alibi) in bf16. Device-side
  the softmax becomes exp(scores) * ea -- one bf16 DVE multiply per score
  tile (2x DVE mode) instead of f32 adds + PE identity matmuls, and the
  alibi HBM traffic halves (bf16 instead of f32).
- Projections are post-scaled: ps = W_s@x_raw - mu (x) wbar accumulates in
  PSUM (raw, unnormalized rhs), then one Pool-engine multiply by the
  broadcast 1/sigma applies the LN scale. No per-chunk input scaling.
- LN stats: x and x^2 streams are tree-folded 8->2 chunks on DVE (bf16 2x),
  then a onehot ones-matmul on PE reduces the remaining 2x128 channels,
  accumulating all token tiles into one [4, TT] PSUM tile per stat.
- V is built directly in [key, dh] natural layout by flipping the matmul
  (lhsT = cT token block, rhs = Wv chunk), so no PE transposes / vaug
  copies; the 1/sigma scale rides the PSUM->SBUF Act copy as a per-key
  scale vector (obtained by tiny PE transposes of the stat rows).
- The two heads' scores go into one 2-bank PSUM tile so one Act exp
  covers [128, 1024]; the softmax denominator rides the AV matmul as a
  ones-column of V (row 64 of the 65-row AV output).
- Output projection PSUM is staged to SBUF by the (otherwise idle) Pool
  engine; bo is added on the host during the gather.
Host gather: sum the 8 partial [dout, tok] projections, add bo, transpose.
"""

import os
import sys

for _p in ("/opt/trn_rl_repo", "/root/.axon_site/_ro/trn_rl_repo"):
    if os.path.isdir(_p) and _p not in sys.path:
        sys.path.insert(0, _p)

import numpy as np
import ml_dtypes

import concourse.bass as bass
import concourse.tile as tile
from concourse import bacc, mybir
from concourse.masks import make_identity

BF16 = ml_dtypes.bfloat16

HEADS = 16
N_CORES = 8
H_PER_CORE = HEADS // N_CORES  # 2
DH = 64
LN_EPS = 1e-5

B = 2
N_TOK = 2048
D = 1024

QT = 512            # query tile (free dim of scores matmuls)
KT = 128            # key tile (partition dim of scoresT)
TT = 512            # token tile for LN/projection phase
N_DT = D // 128     # 8 contraction tiles of 128 over d


def build_program(n_tok=N_TOK, with_pbias=False):
    """Build the single-core SPMD Bass program. Returns nc."""
    nc = bacc.Bacc("TRN2")
    f32 = mybir.dt.float32
    f32r = mybir.dt.float32r
    bf16 = mybir.dt.bfloat16
    AF = mybir.ActivationFunctionType
    ALU = mybir.AluOpType

    n_tt = n_tok // TT          # token tiles per batch
    n_qt = n_tok // QT          # query tiles per batch
    n_kt = n_tok // KT          # key tiles per batch

    # ---- DRAM parameters (per-core shards, host-prepped) ----
    xT = nc.declare_dram_parameter("xT", [B, D, n_tok], bf16, isOutput=False)
    cT = nc.declare_dram_parameter("cT", [B, D, n_tok], bf16, isOutput=False)
    # exp(alibi) transposed: [h, key, q], bf16
    eaT = nc.declare_dram_parameter(
        "eaT", [H_PER_CORE, n_tok, n_tok], bf16, isOutput=False)
    wqT = nc.declare_dram_parameter("wqT", [D, 128], bf16, isOutput=False)
    wkT = nc.declare_dram_parameter("wkT", [D, 128], bf16, isOutput=False)
    wvT = nc.declare_dram_parameter("wvT", [D, 128], bf16, isOutput=False)
    # rows: -wbar_q, -wbar_k, -wbar_v   (sum over d of the scaled weights)
    wbar = nc.declare_dram_parameter("wbar", [3, 128], bf16, isOutput=False)
    woT = nc.declare_dram_parameter("woT", [128, D], bf16, isOutput=False)
    # host-computed LN stats: mean rows (bf16), 1/sigma rows (f32),
    # per-key 1/sigma columns for the exp scale (f32)
    mrow = nc.declare_dram_parameter("mrow", [2, B, n_tok], bf16, isOutput=False)
    irow = nc.declare_dram_parameter("irow", [2, B, n_tok], f32, isOutput=False)
    icol = nc.declare_dram_parameter("icol", [B, 128, n_tok // 128], f32,
                                     isOutput=False)
    if with_pbias:
        # rows: Wq@ln_b*scale, Wk@ln_b, Wv@ln_b
        pbias = nc.declare_dram_parameter("pbias", [3, 128], bf16, isOutput=False)
        srow = nc.declare_dram_parameter("srow", [2, B, n_tok], bf16,
                                         isOutput=False)

    outT = nc.declare_dram_parameter(
        "outT", [D, B * n_tok], f32, isOutput=True)

    xT_r = xT.rearrange("b (dt p) n -> b p dt n", p=128)
    cT_r = cT.rearrange("b (dt p) n -> b p dt n", p=128)
    woT_r = woT.rearrange("c (dt n) -> c dt n", n=128)
    outT_r = outT.rearrange("(dt p) n -> p dt n", p=128)

    with tile.TileContext(nc) as tc:
        with tc.tile_pool(name="const", bufs=1) as const_pool, \
             tc.tile_pool(name="rowp", bufs=2) as rowp:
            ident_b = const_pool.tile([128, 128], bf16, name="ident_b")
            make_identity(nc, ident_b)
            icol_sb = const_pool.tile([128, B, n_tok // 128], f32,
                                      name="icol_sb")
            nc.sync.dma_start(out=icol_sb,
                              in_=icol.rearrange("b p k -> p b k"))

            wq_sb = const_pool.tile([128, N_DT, 128], bf16, name="wq_sb")
            wk_sb = const_pool.tile([128, N_DT, 128], bf16, name="wk_sb")
            wv_sb = const_pool.tile([128, N_DT, 128], bf16, name="wv_sb")
            nc.sync.dma_start(out=wq_sb, in_=wqT.rearrange("(dt p) c -> p dt c", p=128))
            nc.sync.dma_start(out=wk_sb, in_=wkT.rearrange("(dt p) c -> p dt c", p=128))
            nc.sync.dma_start(out=wv_sb, in_=wvT.rearrange("(dt p) c -> p dt c", p=128))
            wbar_sb = const_pool.tile([1, 3, 128], bf16, name="wbar_sb")
            nc.sync.dma_start(out=wbar_sb, in_=wbar[None, :, :])
            wo_sb = const_pool.tile([128, N_DT, 128], bf16, name="wo_sb")
            nc.sync.dma_start(out=wo_sb, in_=woT_r)
            if with_pbias:
                pb_sb = const_pool.tile([1, 3, 128], bf16, name="pb_sb")
                nc.sync.dma_start(out=pb_sb, in_=pbias[None, :, :])

            # persistent activations: q/k transposed f32 (f32r for PE speed)
            qT_sb = const_pool.tile([128, B, n_tok], f32r, name="qT_sb")
            kT_sb = const_pool.tile([128, B, n_tok], f32r, name="kT_sb")
            vT_sb = const_pool.tile([128, B, n_tok], bf16, name="vT_sb")
            # v natural (+ones col): [key(128), b*n_kt*h, 66]
            vaug_sb = const_pool.tile(
                [128, B * n_kt * H_PER_CORE, 66], bf16, name="vaug_sb")
            nc.vector.memset(vaug_sb[:, :, 64:65], 1.0)

            def vaug_idx(b, kt, h):
                return (b * n_kt + kt) * H_PER_CORE + h


            # ============ Phase A helpers (host LN stats) ========
            raw_p = tc.alloc_tile_pool(name="raw_p", bufs=n_tt + 2)
            rowp = tc.alloc_tile_pool(name="rowp", bufs=2)
            isb_p = tc.alloc_tile_pool(name="isb_p", bufs=3)
            ps_pool = tc.alloc_tile_pool(name="ps_pool", bufs=2, space="PSUM")

            def emit_A(src_i, b, src_r):
                """LN-apply + projections for one (src, b). Q and V are
                post-scaled by 1/sigma (DVE); K stays unnormalized -- the
                per-key 1/sigma rides the phase-B exp as its per-partition
                scale vector. V is then transposed to [key, dh] natural
                layout through the PE."""
                raws = []
                for u in range(n_tt):
                    raw = raw_p.tile([128, N_DT, TT], bf16, tag="raw",
                                     name="raw")
                    raws.append(raw)
                    nc.sync.dma_start(
                        out=raw, in_=src_r[b, :, :, u * TT:(u + 1) * TT])
                plist = ((0, wq_sb, qT_sb),) if src_i == 0 else \
                        ((1, wk_sb, kT_sb), (2, wv_sb, vT_sb))
                for u in range(n_tt):
                    t_sl = slice(u * TT, (u + 1) * TT)
                    m_row = rowp.tile([1, TT], bf16, tag="m_row",
                                      name="m_row")
                    nc.sync.dma_start(out=m_row, in_=mrow[None, src_i, b, t_sl])
                    if with_pbias:
                        s_row = rowp.tile([1, TT], bf16, tag="s_row",
                                          name="s_row")
                        nc.sync.dma_start(out=s_row,
                                          in_=srow[None, src_i, b, t_sl])
                    isb = None
                    for wi, w_sb, dst in plist:
                        if wi != 1 and isb is None:
                            i_row = rowp.tile([1, TT], f32, tag="i_row",
                                              name="i_row")
                            nc.sync.dma_start(
                                out=i_row, in_=irow[None, src_i, b, t_sl])
                            isb = isb_p.tile([128, TT], f32, tag="isb",
                                             name="isb")
                            nc.gpsimd.partition_broadcast(isb, i_row)
                        ps = ps_pool.tile([128, TT], f32, tag="ps", name="ps")
                        for dt in range(N_DT):
                            nc.tensor.matmul(
                                ps, w_sb[:, dt, :], raws[u][:, dt, :],
                                start=(dt == 0), stop=False)
                        nc.tensor.matmul(
                            ps, wbar_sb[:, wi, :], m_row,
                            start=False, stop=not with_pbias)
                        if with_pbias:
                            nc.tensor.matmul(
                                ps, pb_sb[:, wi, :], s_row,
                                start=False, stop=True)
                        dsl = dst[:, b, t_sl]
                        if wi == 1:
                            nc.scalar.activation(
                                out=dsl, in_=ps, func=AF.Copy,
                                bias=0.0, scale=1.0)
                        else:
                            nc.vector.tensor_mul(dsl, ps, isb)
                if src_i == 1:
                    for kt in range(n_kt):
                        # vt shares the ps tag (sequential after the projs)
                        vt = ps_pool.tile([128, 128], bf16, tag="ps",
                                          name="vt")
                        nc.tensor.transpose(
                            vt, vT_sb[:, b, kt * KT:(kt + 1) * KT], ident_b)
                        i0 = vaug_idx(b, kt, 0)
                        nc.scalar.activation(
                            out=vaug_sb[:, i0:i0 + 2, 0:64],
                            in_=vt.rearrange("p (h c) -> p h c", h=2),
                            func=AF.Copy, bias=0.0, scale=1.0)

            # ============ Phase B: attention ========
            for b in range(B):
                emit_A(0, b, xT_r)
                emit_A(1, b, cT_r)
            for _pool in (ps_pool, isb_p, rowp, raw_p):
                _pool.release()
            ea_p = tc.alloc_tile_pool(name="ea_p", bufs=8)
            ex_p = tc.alloc_tile_pool(name="ex_p", bufs=4)
            den_p = tc.alloc_tile_pool(name="den_p", bufs=2)
            fo_p = tc.alloc_tile_pool(name="fo_p", bufs=1)
            sc_ps = tc.alloc_tile_pool(name="sc_ps", bufs=1, space="PSUM")
            av_ps = tc.alloc_tile_pool(name="av_ps", bufs=1, space="PSUM")

            state = {"fo": [None, None]}

            HD = N_DT // 2

            def emit_unit(qt, b, dt, o_sb):
                """One output-projection unit: dt-chunk matmul + copy.
                fo is staged in half-tiles (4 dt chunks) to halve SBUF."""
                if dt % HD == 0:
                    state["fo"][b] = fo_p.tile(
                        [128, HD, QT], f32, tag="fo", bufs=2, name="fo")
                fo = state["fo"][b]
                fp = sc_ps.tile([128, 2, QT], f32, tag=f"sc_k{dt % 2}",
                                name="fp")
                nc.tensor.matmul(fp[:, 0, :], wo_sb[:, dt, :], o_sb,
                                 start=True, stop=True)
                if dt % 2 == 0:
                    nc.scalar.activation(
                        out=fo[:, dt % HD, :], in_=fp[:, 0, :],
                        func=AF.Copy, bias=0.0, scale=1.0)
                else:
                    nc.vector.tensor_copy(fo[:, dt % HD, :], fp[:, 0, :])
                if dt % HD == HD - 1:
                    nc.sync.dma_start(
                        out=outT_r[:, (dt // HD) * HD:(dt // HD + 1) * HD,
                                   b * n_tok + qt * QT:
                                   b * n_tok + (qt + 1) * QT],
                        in_=fo)

            pending = []

            def emit_pass(qt):
                q_sl = slice(qt * QT, (qt + 1) * QT)
                av = [[av_ps.tile([65, QT], f32, tag=f"av{b}{h}",
                                  name=f"av{b}{h}")
                       for h in range(H_PER_CORE)] for b in range(B)]
                for kt in range(n_kt):
                    if pending:
                        pqt, pb, osb = pending[0]
                        emit_unit(pqt, pb, kt % N_DT, osb)
                        if kt % N_DT == N_DT - 1:
                            pending.pop(0)
                    k_sl = slice(kt * KT, (kt + 1) * KT)
                    ea = ea_p.tile([128, 2, QT], bf16, tag="ea", name="ea")
                    nc.sync.dma_start(
                        out=ea,
                        in_=eaT[:, k_sl, q_sl].rearrange("h p n -> p h n"))
                    for b in range(B):
                        sc2 = sc_ps.tile([128, 2, QT], f32, tag=f"sc_k{b}",
                                         name="sc2")
                        for h in range(H_PER_CORE):
                            c_sl = slice(h * 64, (h + 1) * 64)
                            nc.tensor.matmul(
                                sc2[:, h, :], kT_sb[c_sl, b, k_sl],
                                qT_sb[c_sl, b, q_sl],
                                start=True, stop=True,
                                tile_position=(h * 64, 0))
                        ex_raw = ex_p.tile([128, 2, QT], bf16, tag="ex_raw",
                                           name="ex_raw")
                        nc.scalar.activation(
                            out=ex_raw, in_=sc2, func=AF.Exp, bias=0.0,
                            scale=icol_sb[:, b, kt:kt + 1])
                        ex = ex_p.tile([128, 2, QT], bf16, tag="ex",
                                       name="ex")
                        nc.vector.tensor_mul(ex, ex_raw, ea)
                        for h in range(H_PER_CORE):
                            nc.tensor.matmul(
                                av[b][h], vaug_sb[:, vaug_idx(b, kt, h), 0:65],
                                ex[:, h, :],
                                start=(kt == 0), stop=(kt == n_kt - 1))
                for b in range(B):
                    # normalize fast (frees the av banks for the next qt)
                    o_sb = den_p.tile([128, QT], bf16, tag=f"o_sb{b}",
                                      name="o_sb")
                    for h in range(H_PER_CORE):
                        den = den_p.tile([1, QT], f32, tag=f"den{h}",
                                         name="den")
                        nc.vector.tensor_copy(den, av[b][h][64:65, :])
                        rden = den_p.tile([1, QT], f32, tag=f"rden{h}",
                                          name="rden")
                        nc.vector.reciprocal_approx_fast(rden, den)
                        rb = den_p.tile([64, QT], f32, tag=f"rb{h}",
                                        name="rb")
                        nc.gpsimd.partition_broadcast(rb, rden)
                        nc.vector.tensor_mul(o_sb[h * 64:(h + 1) * 64, :],
                                             av[b][h][0:64, :], rb)
                    pending.append((qt, b, o_sb))

            for qt in range(n_qt):
                emit_pass(qt)
            for pqt, pb, osb in pending:
                for dt in range(N_DT):
                    emit_unit(pqt, pb, dt, osb)
            for _pool in (av_ps, sc_ps, fo_p, den_p, ex_p, ea_p):
                _pool.release()
    nc.compile()
    return nc


_NC_CACHE = {}


def _get_program(n_tok=N_TOK, with_pbias=False):
    key = (n_tok, with_pbias)
    if key not in _NC_CACHE:
        _NC_CACHE[key] = build_program(n_tok, with_pbias)
    return _NC_CACHE[key]


def _prep_in_maps(x, context, alibi, Wq, Wk, Wv, Wo, bo, ln_w, ln_b):
    b, n, d = x.shape
    scale = (d // HEADS) ** -0.5

    x = np.asarray(x, dtype=np.float32)
    context = np.asarray(context, dtype=np.float32)
    alibi = np.asarray(alibi, dtype=np.float32)
    Wq, Wk, Wv, Wo = (np.asarray(w, dtype=np.float32) for w in (Wq, Wk, Wv, Wo))
    ln_w = np.asarray(ln_w, dtype=np.float32)
    ln_b = np.asarray(ln_b, dtype=np.float32)

    xT = np.ascontiguousarray(x.transpose(0, 2, 1)).astype(BF16)
    cT = np.ascontiguousarray(context.transpose(0, 2, 1)).astype(BF16)
    # exp(alibi), transposed to [h, key, q], bf16
    eaT_full = np.exp(alibi[0]).transpose(0, 2, 1)

    with_pbias = bool(np.any(ln_b != 0.0))

    # LN stats on the bf16-rounded inputs (what the device streams)
    def stats(t):
        tf = np.asarray(t, dtype=np.float32)
        mu = tf.mean(-1)                                    # [b, n]
        var = tf.var(-1)
        ivs = 1.0 / np.sqrt(var + LN_EPS)
        return mu, ivs
    mu_x, iv_x = stats(xT.transpose(0, 2, 1))
    mu_c, iv_c = stats(cT.transpose(0, 2, 1))
    mrow = np.stack([mu_x, mu_c]).astype(BF16)              # [2, B, n]
    irow = np.stack([iv_x, iv_c]).astype(np.float32)        # [2, B, n]
    icol = np.ascontiguousarray(
        iv_c.reshape(b, n // 128, 128).transpose(0, 2, 1)).astype(np.float32)
    srow = np.stack([1.0 / iv_x, 1.0 / iv_c]).astype(BF16)

    in_maps = []
    for ci in range(N_CORES):
        h0 = ci * H_PER_CORE
        cs = slice(h0 * DH, (h0 + H_PER_CORE) * DH)  # this core's 128 channels

        wq_s = (Wq[cs] * ln_w[None, :]) * scale          # [128, d]
        wk_s = Wk[cs] * ln_w[None, :]
        wv_s = Wv[cs] * ln_w[None, :]
        wbar = np.stack([
            -wq_s.sum(axis=1), -wk_s.sum(axis=1), -wv_s.sum(axis=1)])

        m = {
            "xT": xT,
            "cT": cT,
            "eaT": np.ascontiguousarray(eaT_full[h0:h0 + H_PER_CORE]).astype(BF16),
            "wqT": np.ascontiguousarray(wq_s.T).astype(BF16),
            "wkT": np.ascontiguousarray(wk_s.T).astype(BF16),
            "wvT": np.ascontiguousarray(wv_s.T).astype(BF16),
            "wbar": wbar.astype(BF16),
            "woT": np.ascontiguousarray(Wo[:, cs].T).astype(BF16),
            "mrow": mrow,
            "irow": irow,
            "icol": icol,
        }
        if with_pbias:
            m["pbias"] = np.stack([
                (Wq[cs] @ ln_b) * scale, Wk[cs] @ ln_b,
                Wv[cs] @ ln_b]).astype(BF16)
            m["srow"] = srow
        in_maps.append(m)
    return in_maps, with_pbias


def _gather(results, b, n, d, bo):
    acc = np.zeros((d, b * n), dtype=np.float32)
    for r in results:
        acc += r["outT"].astype(np.float32)
    acc += np.asarray(bo, dtype=np.float32)[:, None]
    return np.ascontiguousarray(
        acc.reshape(d, b, n).transpose(1, 2, 0)).astype(np.float32)


def kernel(**inputs):
    from concourse.bass_utils import run_bass_kernel_spmd
    x = inputs["x"]
    b, n, d = x.shape
    in_maps, with_pbias = _prep_in_maps(**inputs)
    nc = _get_program(n, with_pbias)
    res = run_bass_kernel_spmd(nc, in_maps, list(range(N_CORES)))
    return _gather(res.results, b, n, d, inputs["bo"])


def run_profiled(inputs, trace=True):
    from concourse.bass_utils import run_bass_kernel_spmd
    x = inputs["x"]
    b, n, d = x.shape
    in_maps, with_pbias = _prep_in_maps(**inputs)
    nc = _get_program(n, with_pbias)
    res = run_bass_kernel_spmd(nc, in_maps, list(range(N_CORES)), trace=trace)
    return _gather(res.results, b, n, d, inputs["bo"]), res


# revision 24
# speedup vs baseline: 1.1252x; 1.0354x over previous
"""CrossAttention kernel for 8 Trainium2 NeuronCores (Bass/Tile).

Sharding: tensor-parallel over heads. Core i handles heads {2i, 2i+1} for
both batch elements (128 channels).

v2 design notes (vs the v1 baseline):
- alibi is exponentiated on the host: ea = exp(alibi) in bf16. Device-side
  the softmax becomes exp(scores) * ea -- one bf16 DVE multiply per score
  tile (2x DVE mode) instead of f32 adds + PE identity matmuls, and the
  alibi HBM traffic halves (bf16 instead of f32).
- Projections are post-scaled: ps = W_s@x_raw - mu (x) wbar accumulates in
  PSUM (raw, unnormalized rhs), then one Pool-engine multiply by the
  broadcast 1/sigma applies the LN scale. No per-chunk input scaling.
- LN stats: x and x^2 streams are tree-folded 8->2 chunks on DVE (bf16 2x),
  then a onehot ones-matmul on PE reduces the remaining 2x128 channels,
  accumulating all token tiles into one [4, TT] PSUM tile per stat.
- V is built directly in [key, dh] natural layout by flipping the matmul
  (lhsT = cT token block, rhs = Wv chunk), so no PE transposes / vaug
  copies; the 1/sigma scale rides the PSUM->SBUF Act copy as a per-key
  scale vector (obtained by tiny PE transposes of the stat rows).
- The two heads' scores go into one 2-bank PSUM tile so one Act exp
  covers [128, 1024]; the softmax denominator rides the AV matmul as a
  ones-column of V (row 64 of the 65-row AV output).
- Output projection PSUM is staged to SBUF by the (otherwise idle) Pool
  engine; bo is added on the host during the gather.
Host gather: sum the 8 partial [dout, tok] projections, add bo, transpose.
"""

import os
import sys

for _p in ("/opt/trn_rl_repo", "/root/.axon_site/_ro/trn_rl_repo"):
    if os.path.isdir(_p) and _p not in sys.path:
        sys.path.insert(0, _p)

import numpy as np
import ml_dtypes

import concourse.bass as bass
import concourse.tile as tile
from concourse import bacc, mybir
from concourse.masks import make_identity

BF16 = ml_dtypes.bfloat16

HEADS = 16
N_CORES = 8
H_PER_CORE = HEADS // N_CORES  # 2
DH = 64
LN_EPS = 1e-5

B = 2
N_TOK = 2048
D = 1024

QT = 512            # query tile (free dim of scores matmuls)
KT = 128            # key tile (partition dim of scoresT)
TT = 512            # token tile for LN/projection phase
N_DT = D // 128     # 8 contraction tiles of 128 over d


def build_program(n_tok=N_TOK, with_pbias=False):
    """Build the single-core SPMD Bass program. Returns nc."""
    nc = bacc.Bacc("TRN2")
    f32 = mybir.dt.float32
    f32r = mybir.dt.float32r
    bf16 = mybir.dt.bfloat16
    AF = mybir.ActivationFunctionType
    ALU = mybir.AluOpType

    n_tt = n_tok // TT          # token tiles per batch
    n_qt = n_tok // QT          # query tiles per batch
    n_kt = n_tok // KT          # key tiles per batch

    # ---- DRAM parameters (per-core shards, host-prepped) ----
    xT = nc.declare_dram_parameter("xT", [B, D, n_tok], bf16, isOutput=False)
    cT = nc.declare_dram_parameter("cT", [B, D, n_tok], bf16, isOutput=False)
    # exp(alibi) transposed: [h, key, q], bf16
    eaT = nc.declare_dram_parameter(
        "eaT", [H_PER_CORE, n_tok, n_tok], bf16, isOutput=False)
    wqT = nc.declare_dram_parameter("wqT", [D, 128], bf16, isOutput=False)
    wkT = nc.declare_dram_parameter("wkT", [D, 128], bf16, isOutput=False)
    wvT = nc.declare_dram_parameter("wvT", [D, 128], bf16, isOutput=False)
    # rows: -wbar_q, -wbar_k, -wbar_v   (sum over d of the scaled weights)
    wbar = nc.declare_dram_parameter("wbar", [3, 128], bf16, isOutput=False)
    woT = nc.declare_dram_parameter("woT", [128, D], bf16, isOutput=False)
    # host-computed LN stats: mean rows (bf16), 1/sigma rows (f32),
    # per-key 1/sigma columns for the exp scale (f32)
    mrow = nc.declare_dram_parameter("mrow", [2, B, n_tok], bf16, isOutput=False)
    irow = nc.declare_dram_parameter("irow", [2, B, n_tok], f32, isOutput=False)
    icol = nc.declare_dram_parameter("icol", [B, 128, n_tok // 128], f32,
                                     isOutput=False)
    if with_pbias:
        # rows: Wq@ln_b*scale, Wk@ln_b, Wv@ln_b
        pbias = nc.declare_dram_parameter("pbias", [3, 128], bf16, isOutput=False)
        srow = nc.declare_dram_parameter("srow", [2, B, n_tok], bf16,
                                         isOutput=False)

    outT = nc.declare_dram_parameter(
        "outT", [D, B * n_tok], f32, isOutput=True)

    xT_r = xT.rearrange("b (dt p) n -> b p dt n", p=128)
    cT_r = cT.rearrange("b (dt p) n -> b p dt n", p=128)
    woT_r = woT.rearrange("c (dt n) -> c dt n", n=128)
    outT_r = outT.rearrange("(dt p) n -> p dt n", p=128)

    with tile.TileContext(nc) as tc:
        with tc.tile_pool(name="const", bufs=1) as const_pool, \
             tc.tile_pool(name="rowp", bufs=2) as rowp:
            ident_b = const_pool.tile([128, 128], bf16, name="ident_b")
            make_identity(nc, ident_b)
            icol_sb = const_pool.tile([128, B, n_tok // 128], f32,
                                      name="icol_sb")
            nc.sync.dma_start(out=icol_sb,
                              in_=icol.rearrange("b p k -> p b k"))

            wq_sb = const_pool.tile([128, N_DT, 128], bf16, name="wq_sb")
            wk_sb = const_pool.tile([128, N_DT, 128], bf16, name="wk_sb")
            wv_sb = const_pool.tile([128, N_DT, 128], bf16, name="wv_sb")
            nc.sync.dma_start(out=wq_sb, in_=wqT.rearrange("(dt p) c -> p dt c", p=128))
            nc.sync.dma_start(out=wk_sb, in_=wkT.rearrange("(dt p) c -> p dt c", p=128))
            nc.sync.dma_start(out=wv_sb, in_=wvT.rearrange("(dt p) c -> p dt c", p=128))
            wbar_sb = const_pool.tile([1, 3, 128], bf16, name="wbar_sb")
            nc.sync.dma_start(out=wbar_sb, in_=wbar[None, :, :])
            wo_sb = const_pool.tile([128, N_DT, 128], bf16, name="wo_sb")
            nc.sync.dma_start(out=wo_sb, in_=woT_r)
            if with_pbias:
                pb_sb = const_pool.tile([1, 3, 128], bf16, name="pb_sb")
                nc.sync.dma_start(out=pb_sb, in_=pbias[None, :, :])

            # persistent activations: q/k transposed f32 (f32r for PE speed)
            qT_sb = const_pool.tile([128, B, n_tok], f32r, name="qT_sb")
            kT_sb = const_pool.tile([128, B, n_tok], f32r, name="kT_sb")
            vT_sb = const_pool.tile([128, B, n_tok], bf16, name="vT_sb")
            # v natural (+ones col): [key(128), b*n_kt*h, 66]
            vaug_sb = const_pool.tile(
                [128, B * n_kt * H_PER_CORE, 66], bf16, name="vaug_sb")
            nc.vector.memset(vaug_sb[:, :, 64:65], 1.0)

            def vaug_idx(b, kt, h):
                return (b * n_kt + kt) * H_PER_CORE + h


            # ============ Phase A helpers (host LN stats) ========
            raw_p = tc.alloc_tile_pool(name="raw_p", bufs=n_tt + 2)
            rowp = tc.alloc_tile_pool(name="rowp", bufs=2)
            isb_p = tc.alloc_tile_pool(name="isb_p", bufs=3)
            ps_pool = tc.alloc_tile_pool(name="ps_pool", bufs=2, space="PSUM")

            def emit_A(src_i, b, src_r):
                """LN-apply + projections for one (src, b). Q and V are
                post-scaled by 1/sigma (DVE); K stays unnormalized -- the
                per-key 1/sigma rides the phase-B exp as its per-partition
                scale vector. V is then transposed to [key, dh] natural
                layout through the PE."""
                raws = []
                for u in range(n_tt):
                    raw = raw_p.tile([128, N_DT, TT], bf16, tag="raw",
                                     name="raw")
                    raws.append(raw)
                    nc.sync.dma_start(
                        out=raw, in_=src_r[b, :, :, u * TT:(u + 1) * TT])
                plist = ((0, wq_sb, qT_sb),) if src_i == 0 else \
                        ((1, wk_sb, kT_sb), (2, wv_sb, vT_sb))
                m_rows, i_rows, s_rows = [], [], []
                for u in range(n_tt):
                    t_sl = slice(u * TT, (u + 1) * TT)
                    m_row = rowp.tile([1, TT], bf16, tag="m_row", bufs=5,
                                      name="m_row")
                    nc.sync.dma_start(out=m_row,
                                      in_=mrow[None, src_i, b, t_sl])
                    m_rows.append(m_row)
                    i_row = rowp.tile([1, TT], f32, tag="i_row", bufs=5,
                                      name="i_row")
                    nc.sync.dma_start(out=i_row,
                                      in_=irow[None, src_i, b, t_sl])
                    i_rows.append(i_row)
                    if with_pbias:
                        s_row = rowp.tile([1, TT], bf16, tag="s_row", bufs=5,
                                          name="s_row")
                        nc.sync.dma_start(out=s_row,
                                          in_=srow[None, src_i, b, t_sl])
                        s_rows.append(s_row)
                for u in range(n_tt):
                    t_sl = slice(u * TT, (u + 1) * TT)
                    m_row = m_rows[u]
                    if with_pbias:
                        s_row = s_rows[u]
                    isb = None
                    for wi, w_sb, dst in plist:
                        if wi != 1 and isb is None:
                            isb = isb_p.tile([128, TT], f32, tag="isb",
                                             name="isb")
                            nc.gpsimd.partition_broadcast(isb, i_rows[u])
                        ps = ps_pool.tile([128, TT], f32, tag="ps", name="ps")
                        for dt in range(N_DT):
                            nc.tensor.matmul(
                                ps, w_sb[:, dt, :], raws[u][:, dt, :],
                                start=(dt == 0), stop=False)
                        nc.tensor.matmul(
                            ps, wbar_sb[:, wi, :], m_row,
                            start=False, stop=not with_pbias)
                        if with_pbias:
                            nc.tensor.matmul(
                                ps, pb_sb[:, wi, :], s_row,
                                start=False, stop=True)
                        dsl = dst[:, b, t_sl]
                        if wi == 1:
                            nc.scalar.activation(
                                out=dsl, in_=ps, func=AF.Copy,
                                bias=0.0, scale=1.0)
                        else:
                            nc.vector.tensor_mul(dsl, ps, isb)
                if src_i == 1:
                    for kt in range(n_kt):
                        # vt shares the ps tag (sequential after the projs)
                        vt = ps_pool.tile([128, 128], bf16, tag="ps",
                                          name="vt")
                        nc.tensor.transpose(
                            vt, vT_sb[:, b, kt * KT:(kt + 1) * KT], ident_b)
                        i0 = vaug_idx(b, kt, 0)
                        nc.scalar.activation(
                            out=vaug_sb[:, i0:i0 + 2, 0:64],
                            in_=vt.rearrange("p (h c) -> p h c", h=2),
                            func=AF.Copy, bias=0.0, scale=1.0)

            # ============ Phase B: attention ========
            for b in range(B):
                emit_A(0, b, xT_r)
                emit_A(1, b, cT_r)
            for _pool in (ps_pool, isb_p, rowp, raw_p):
                _pool.release()
            ea_p = tc.alloc_tile_pool(name="ea_p", bufs=8)
            ex_p = tc.alloc_tile_pool(name="ex_p", bufs=4)
            den_p = tc.alloc_tile_pool(name="den_p", bufs=2)
            fo_p = tc.alloc_tile_pool(name="fo_p", bufs=1)
            sc_ps = tc.alloc_tile_pool(name="sc_ps", bufs=1, space="PSUM")
            av_ps = tc.alloc_tile_pool(name="av_ps", bufs=1, space="PSUM")

            state = {"fo": [None, None]}

            HD = N_DT // 2

            def emit_unit(qt, b, dt, o_sb):
                """One output-projection unit: dt-chunk matmul + copy.
                fo is staged in half-tiles (4 dt chunks) to halve SBUF."""
                if dt % HD == 0:
                    state["fo"][b] = fo_p.tile(
                        [128, HD, QT], f32, tag="fo", bufs=2, name="fo")
                fo = state["fo"][b]
                fp = sc_ps.tile([128, 2, QT], f32, tag=f"sc_k{dt % 2}",
                                name="fp")
                nc.tensor.matmul(fp[:, 0, :], wo_sb[:, dt, :], o_sb,
                                 start=True, stop=True)
                if dt % 2 == 0:
                    nc.scalar.activation(
                        out=fo[:, dt % HD, :], in_=fp[:, 0, :],
                        func=AF.Copy, bias=0.0, scale=1.0)
                else:
                    nc.vector.tensor_copy(fo[:, dt % HD, :], fp[:, 0, :])
                if dt % HD == HD - 1:
                    nc.sync.dma_start(
                        out=outT_r[:, (dt // HD) * HD:(dt // HD + 1) * HD,
                                   b * n_tok + qt * QT:
                                   b * n_tok + (qt + 1) * QT],
                        in_=fo)

            pending = []

            def emit_pass(qt):
                q_sl = slice(qt * QT, (qt + 1) * QT)
                av = [[av_ps.tile([65, QT], f32, tag=f"av{b}{h}",
                                  name=f"av{b}{h}")
                       for h in range(H_PER_CORE)] for b in range(B)]
                for kt in range(n_kt):
                    if pending:
                        pqt, pb, osb = pending[0]
                        emit_unit(pqt, pb, kt % N_DT, osb)
                        if kt % N_DT == N_DT - 1:
                            pending.pop(0)
                    k_sl = slice(kt * KT, (kt + 1) * KT)
                    ea = ea_p.tile([128, 2, QT], bf16, tag="ea", name="ea")
                    nc.sync.dma_start(
                        out=ea,
                        in_=eaT[:, k_sl, q_sl].rearrange("h p n -> p h n"))
                    for b in range(B):
                        sc2 = sc_ps.tile([128, 2, QT], f32, tag=f"sc_k{b}",
                                         name="sc2")
                        for h in range(H_PER_CORE):
                            c_sl = slice(h * 64, (h + 1) * 64)
                            nc.tensor.matmul(
                                sc2[:, h, :], kT_sb[c_sl, b, k_sl],
                                qT_sb[c_sl, b, q_sl],
                                start=True, stop=True,
                                tile_position=(h * 64, 0))
                        ex_raw = ex_p.tile([128, 2, QT], bf16, tag="ex_raw",
                                           name="ex_raw")
                        nc.scalar.activation(
                            out=ex_raw, in_=sc2, func=AF.Exp, bias=0.0,
                            scale=icol_sb[:, b, kt:kt + 1])
                        ex = ex_p.tile([128, 2, QT], bf16, tag="ex",
                                       name="ex")
                        nc.vector.tensor_mul(ex, ex_raw, ea)
                        for h in range(H_PER_CORE):
                            nc.tensor.matmul(
                                av[b][h], vaug_sb[:, vaug_idx(b, kt, h), 0:65],
                                ex[:, h, :],
                                start=(kt == 0), stop=(kt == n_kt - 1))
                for b in range(B):
                    # normalize fast (frees the av banks for the next qt)
                    o_sb = den_p.tile([128, QT], bf16, tag=f"o_sb{b}",
                                      name="o_sb")
                    for h in range(H_PER_CORE):
                        den = den_p.tile([1, QT], f32, tag=f"den{h}",
                                         name="den")
                        nc.vector.tensor_copy(den, av[b][h][64:65, :])
                        rden = den_p.tile([1, QT], f32, tag=f"rden{h}",
                                          name="rden")
                        nc.vector.reciprocal_approx_fast(rden, den)
                        rb = den_p.tile([64, QT], f32, tag=f"rb{h}",
                                        name="rb")
                        nc.gpsimd.partition_broadcast(rb, rden)
                        nc.vector.tensor_mul(o_sb[h * 64:(h + 1) * 64, :],
                                             av[b][h][0:64, :], rb)
                    pending.append((qt, b, o_sb))

            for qt in range(n_qt):
                emit_pass(qt)
            for pqt, pb, osb in pending:
                for dt in range(N_DT):
                    emit_unit(pqt, pb, dt, osb)
            for _pool in (av_ps, sc_ps, fo_p, den_p, ex_p, ea_p):
                _pool.release()
    nc.compile()
    return nc


_NC_CACHE = {}


def _get_program(n_tok=N_TOK, with_pbias=False):
    key = (n_tok, with_pbias)
    if key not in _NC_CACHE:
        _NC_CACHE[key] = build_program(n_tok, with_pbias)
    return _NC_CACHE[key]


def _prep_in_maps(x, context, alibi, Wq, Wk, Wv, Wo, bo, ln_w, ln_b):
    b, n, d = x.shape
    scale = (d // HEADS) ** -0.5

    x = np.asarray(x, dtype=np.float32)
    context = np.asarray(context, dtype=np.float32)
    alibi = np.asarray(alibi, dtype=np.float32)
    Wq, Wk, Wv, Wo = (np.asarray(w, dtype=np.float32) for w in (Wq, Wk, Wv, Wo))
    ln_w = np.asarray(ln_w, dtype=np.float32)
    ln_b = np.asarray(ln_b, dtype=np.float32)

    xT = np.ascontiguousarray(x.transpose(0, 2, 1)).astype(BF16)
    cT = np.ascontiguousarray(context.transpose(0, 2, 1)).astype(BF16)
    # exp(alibi), transposed to [h, key, q], bf16
    eaT_full = np.exp(alibi[0]).transpose(0, 2, 1)

    with_pbias = bool(np.any(ln_b != 0.0))

    # LN stats on the bf16-rounded inputs (what the device streams)
    def stats(t):
        tf = np.asarray(t, dtype=np.float32)
        mu = tf.mean(-1)                                    # [b, n]
        var = tf.var(-1)
        ivs = 1.0 / np.sqrt(var + LN_EPS)
        return mu, ivs
    mu_x, iv_x = stats(xT.transpose(0, 2, 1))
    mu_c, iv_c = stats(cT.transpose(0, 2, 1))
    mrow = np.stack([mu_x, mu_c]).astype(BF16)              # [2, B, n]
    irow = np.stack([iv_x, iv_c]).astype(np.float32)        # [2, B, n]
    icol = np.ascontiguousarray(
        iv_c.reshape(b, n // 128, 128).transpose(0, 2, 1)).astype(np.float32)
    srow = np.stack([1.0 / iv_x, 1.0 / iv_c]).astype(BF16)

    in_maps = []
    for ci in range(N_CORES):
        h0 = ci * H_PER_CORE
        cs = slice(h0 * DH, (h0 + H_PER_CORE) * DH)  # this core's 128 channels

        wq_s = (Wq[cs] * ln_w[None, :]) * scale          # [128, d]
        wk_s = Wk[cs] * ln_w[None, :]
        wv_s = Wv[cs] * ln_w[None, :]
        wbar = np.stack([
            -wq_s.sum(axis=1), -wk_s.sum(axis=1), -wv_s.sum(axis=1)])

        m = {
            "xT": xT,
            "cT": cT,
            "eaT": np.ascontiguousarray(eaT_full[h0:h0 + H_PER_CORE]).astype(BF16),
            "wqT": np.ascontiguousarray(wq_s.T).astype(BF16),
            "wkT": np.ascontiguousarray(wk_s.T).astype(BF16),
            "wvT": np.ascontiguousarray(wv_s.T).astype(BF16),
            "wbar": wbar.astype(BF16),
            "woT": np.ascontiguousarray(Wo[:, cs].T).astype(BF16),
            "mrow": mrow,
            "irow": irow,
            "icol": icol,
        }
        if with_pbias:
            m["pbias"] = np.stack([
                (Wq[cs] @ ln_b) * scale, Wk[cs] @ ln_b,
                Wv[cs] @ ln_b]).astype(BF16)
            m["srow"] = srow
        in_maps.append(m)
    return in_maps, with_pbias


def _gather(results, b, n, d, bo):
    acc = np.zeros((d, b * n), dtype=np.float32)
    for r in results:
        acc += r["outT"].astype(np.float32)
    acc += np.asarray(bo, dtype=np.float32)[:, None]
    return np.ascontiguousarray(
        acc.reshape(d, b, n).transpose(1, 2, 0)).astype(np.float32)


def kernel(**inputs):
    from concourse.bass_utils import run_bass_kernel_spmd
    x = inputs["x"]
    b, n, d = x.shape
    in_maps, with_pbias = _prep_in_maps(**inputs)
    nc = _get_program(n, with_pbias)
    res = run_bass_kernel_spmd(nc, in_maps, list(range(N_CORES)))
    return _gather(res.results, b, n, d, inputs["bo"])


def run_profiled(inputs, trace=True):
    from concourse.bass_utils import run_bass_kernel_spmd
    x = inputs["x"]
    b, n, d = x.shape
    in_maps, with_pbias = _prep_in_maps(**inputs)
    nc = _get_program(n, with_pbias)
    res = run_bass_kernel_spmd(nc, in_maps, list(range(N_CORES)), trace=trace)
    return _gather(res.results, b, n, d, inputs["bo"]), res
